# revision 2
# baseline (speedup 1.0000x reference)
# GCN layer kernel for Trainium2: out[b] = relu((a[b] @ x[b]) @ W) * mask[b]
#
# Sharding: data-parallel over the batch (graph) dim. B=8 graphs, 8 cores,
# one graph per core; W replicated. Inputs are the FULL tensors; shards are
# sliced host-side and the per-core outputs stacked back together.
#
# Per-core dataflow (a: [2048,2048], x: [2048,512], W: [512,512]):
#   - All matmul operands are bf16 (same PE rate as f32r; rel-err ~4e-3 vs
#     the 2e-2 gate). a/x/W are cast fp32->bf16 during the DMA load itself
#     (SWDGE cast-on-DMA via nc.gpsimd.dma_start), so no engine time is
#     spent on input casts.
#   - a must be contracted over its column index m, which requires aT with
#     m on the partition axis. Instead of 256 PE transposes (~44us of PE
#     time in the previous version), aT is produced by the DMA xbar
#     transpose (SBUF->SBUF, bf16): one dma_start(transpose=True) per
#     128-row strip of a writes straight into the aT chunk tile, with
#     out[p, mi, j] = strip[j, 128*mi + p]. This also removes the
#     PSUM->SBUF transpose copybacks from DVE/ACT.
#   - t^T[f,n] = sum_m x[m,f] * aT[m,n]:  lhsT = x tile, rhs = aT chunk.
#   - out[n,d] = sum_f t^T[f,n] * W[f,d]: lhsT = t^T tile, rhs = W, which
#     lands out in [n,d] layout for a direct store.
#   - mask[n] = any(x[n,:] != 0) via sum(|x|) > 0 on ACT, applied fused
#     into the ReLU (relu(mask * t) == mask * relu(t)).
#
# Queue assignment: cast loads on the gpsimd (SWDGE) queue, xbar
# transposes on the sync HWDGE queue, output stores on the scalar HWDGE
# queue, so the three streams never block each other.
#
# PE does nothing but real matmuls (~70us of back-to-back bf16 MMs), so
# the HAM clock-gate stays open naturally; a short warm-up burst (some
# tied to the first a-strip loads to spread it through the DMA window)
# covers the cold start.

import numpy as np

B, N, F, D = 8, 2048, 512, 512
P = 128
NT = N // P        # 16 row-tiles of n (and of m, since a is square)
FT = F // P        # 4 tiles of f
NCHUNK = 512       # n is processed in chunks of 512 columns
NJ = N // NCHUNK   # 4
NSUB = NCHUNK // P # 4

_CACHE = {}


def _build_nc():
    from contextlib import ExitStack

    from concourse import bacc, mybir, tile
    from concourse.masks import make_identity

    f32 = mybir.dt.float32
    bf16 = mybir.dt.bfloat16
    AF = mybir.ActivationFunctionType

    nc = bacc.Bacc(None)
    a_d = nc.dram_tensor("a", [N, N], f32, kind="ExternalInput")
    x_d = nc.dram_tensor("x", [N, F], f32, kind="ExternalInput")
    w_d = nc.dram_tensor("kernel", [F, D], f32, kind="ExternalInput")
    o_d = nc.dram_tensor("out", [N, D], f32, kind="ExternalOutput")

    with tile.TileContext(nc) as tc, ExitStack() as ctx:
        const = ctx.enter_context(tc.tile_pool(name="const", bufs=1))
        xp = ctx.enter_context(tc.tile_pool(name="xp", bufs=1))
        wp = ctx.enter_context(tc.tile_pool(name="wp", bufs=1))
        abp = ctx.enter_context(tc.tile_pool(name="abp", bufs=8))
        atp = ctx.enter_context(tc.tile_pool(name="atp", bufs=3))
        ttp = ctx.enter_context(tc.tile_pool(name="ttp", bufs=2))
        outp = ctx.enter_context(tc.tile_pool(name="outp", bufs=3))
        scr = ctx.enter_context(tc.tile_pool(name="scr", bufs=2))
        ps_mm = ctx.enter_context(tc.tile_pool(name="ps_mm", bufs=4, space="PSUM"))
        ps_o = ctx.enter_context(tc.tile_pool(name="ps_o", bufs=2, space="PSUM"))
        ps_w = ctx.enter_context(tc.tile_pool(name="ps_w", bufs=2, space="PSUM"))

        ident = const.tile([P, P], bf16)
        make_identity(nc, ident[:])

        def warm(n, lhs=None):
            # bf16 identity matmuls: register as HAM activity, output unused.
            for _ in range(n):
                pw = ps_w.tile([P, P], f32, tag="psw", name="pw")
                nc.tensor.matmul(
                    pw[:],
                    lhsT=ident[:] if lhs is None else lhs,
                    rhs=ident[:],
                    start=True,
                    stop=True,
                )

        warm(24)

        # a: 16 strip cast-loads (fp32 HBM -> bf16 SBUF) on the gpsimd queue
        ab = []
        for j in range(4):
            t = abp.tile([P, N], bf16, tag="ab", name=f"ab{j}")
            nc.gpsimd.dma_start(t[:], a_d[j * P : (j + 1) * P, :])
            ab.append(t)
            # spread HAM warm-up through the load window (dep on this strip)
            warm(6, lhs=t[:, 0:P])

        # x: 4 column-chunk cast-loads; chunk fi feeds mm1's fi-th pass
        x_sb = xp.tile([P, NT, F], bf16)
        for c in range(FT):
            nc.gpsimd.dma_start(
                x_sb[:, :, c * P : (c + 1) * P],
                x_d[:, c * P : (c + 1) * P].rearrange("(o p) f -> p o f", p=P),
            )

        w_sb = wp.tile([P, FT, D], bf16)
        nc.gpsimd.dma_start(w_sb[:], w_d[:].rearrange("(o p) d -> p o d", p=P))

        for j in range(4, NT):
            t = abp.tile([P, N], bf16, tag="ab", name=f"ab{j}")
            nc.gpsimd.dma_start(t[:], a_d[j * P : (j + 1) * P, :])
            ab.append(t)

        # aT chunks via DMA xbar transpose: strip j fills columns
        # [128*(j%4), 128*(j%4)+128) of chunk j//4 with
        # at[p, mi, jj] = strip[jj, 128*mi + p] = aT[128*mi+p, 128*j+jj]
        at = []
        for nj in range(NJ):
            t = atp.tile([P, NT, NCHUNK], bf16, tag="at", name=f"at{nj}")
            at.append(t)
        for j in range(NT):
            nc.sync.dma_start(
                at[j // 4][:, :, (j % 4) * P : (j % 4 + 1) * P],
                ab[j][:],
                transpose=True,
            )

        # mask accumulators; the per-row-tile |x| reductions ride along
        # inside chunk 0's mm1 phase.
        sumabs = const.tile([P, NT], f32)
        mask_sb = const.tile([P, NT], f32)

        cb = 0

        def copyback(dst, src, eng=None):
            nonlocal cb
            if eng is None:
                eng = "v" if cb % 2 == 0 else "s"
                cb += 1
            if eng == "v":
                nc.vector.tensor_copy(dst, src)
            else:
                nc.scalar.copy(dst, src)

        tts = []
        for nj in range(NJ):
            tt_sb = ttp.tile([P, FT, NCHUNK], bf16, tag="tt", name=f"tt{nj}")
            tts.append(tt_sb)
            for fi in range(FT):
                pt = ps_mm.tile([P, NCHUNK], f32, tag="psm", name=f"pt_{nj}_{fi}")
                for mi in range(NT):
                    nc.tensor.matmul(
                        pt[:],
                        lhsT=x_sb[:, mi, fi * P : (fi + 1) * P],
                        rhs=at[nj][:, mi, :],
                        start=(mi == 0),
                        stop=(mi == NT - 1),
                    )
                if nj == 0:
                    for ni in range(fi * 4, fi * 4 + 4):
                        abs_scr = scr.tile([P, F], f32, tag="abs_scr")
                        nc.scalar.activation(
                            abs_scr[:],
                            x_sb[:, ni, :],
                            AF.Abs,
                            accum_out=sumabs[:, ni : ni + 1],
                        )
                copyback(tt_sb[:, fi], pt[:])
            if nj == 0:
                nc.vector.tensor_scalar(
                    mask_sb[:], sumabs[:], 0.0, None, mybir.AluOpType.is_gt
                )

            # out rows for this chunk: accumulate over the 4 f-tiles, then
            # fused relu+mask on ACT, then store on the scalar HWDGE queue.
            for ns in range(NSUB):
                po = ps_o.tile([P, D], f32, tag="pso", name=f"po_{nj}_{ns}")
                for fi in range(FT):
                    nc.tensor.matmul(
                        po[:],
                        lhsT=tt_sb[:, fi, ns * P : (ns + 1) * P],
                        rhs=w_sb[:, fi],
                        start=(fi == 0),
                        stop=(fi == FT - 1),
                    )
                ni = nj * NSUB + ns
                ob = outp.tile([P, D], f32, tag="ob")
                nc.scalar.activation(
                    ob[:], po[:], AF.Relu, scale=mask_sb[:, ni : ni + 1]
                )
                nc.scalar.dma_start(o_d[ni * P : (ni + 1) * P, :], ob[:])

    nc.compile()
    return nc


def get_nc():
    if "nc" not in _CACHE:
        _CACHE["nc"] = _build_nc()
    return _CACHE["nc"]


def kernel(**inputs) -> np.ndarray:
    from concourse.bass_utils import run_bass_kernel_spmd

    x = np.ascontiguousarray(np.asarray(inputs["x"], dtype=np.float32))
    a = np.ascontiguousarray(np.asarray(inputs["a"], dtype=np.float32))
    w = np.ascontiguousarray(np.asarray(inputs["kernel"], dtype=np.float32))
    assert x.shape == (B, N, F) and a.shape == (B, N, N) and w.shape == (F, D)

    nc = get_nc()
    in_maps = [{"a": a[b], "x": x[b], "kernel": w} for b in range(B)]
    res = run_bass_kernel_spmd(nc, in_maps, core_ids=list(range(B)))
    return np.stack([res.results[b]["out"] for b in range(B)], axis=0)


# revision 5
# speedup vs baseline: 1.0940x; 1.0940x over previous
# GCN layer kernel for Trainium2: out[b] = relu((a[b] @ x[b]) @ W) * mask[b]
#
# Sharding: data-parallel over the batch (graph) dim. B=8 graphs, 8 cores,
# one graph per core; W replicated. Inputs are the FULL tensors; shards are
# sliced host-side and the per-core outputs stacked back together.
#
# Per-core dataflow (a: [2048,2048], x: [2048,512], W: [512,512]):
#   - All matmul operands are bf16 (same PE rate as f32r; rel-err ~4e-3 vs
#     the 2e-2 gate). a/x/W are cast fp32->bf16 during the DMA load itself
#     (SWDGE cast-on-DMA via nc.gpsimd.dma_start), so no engine time is
#     spent on input casts.
#   - a must be contracted over its column index m, which requires aT with
#     m on the partition axis. Instead of 256 PE transposes (~44us of PE
#     time in the previous version), aT is produced by the DMA xbar
#     transpose (SBUF->SBUF, bf16): one dma_start(transpose=True) per
#     128-row strip of a writes straight into the aT chunk tile, with
#     out[p, mi, j] = strip[j, 128*mi + p]. This also removes the
#     PSUM->SBUF transpose copybacks from DVE/ACT.
#   - t^T[f,n] = sum_m x[m,f] * aT[m,n]:  lhsT = x tile, rhs = aT chunk.
#   - out[n,d] = sum_f t^T[f,n] * W[f,d]: lhsT = t^T tile, rhs = W, which
#     lands out in [n,d] layout for a direct store.
#   - mask[n] = any(x[n,:] != 0) via sum(|x|) > 0 on ACT, applied fused
#     into the ReLU (relu(mask * t) == mask * relu(t)).
#
# Queue assignment: a-strip cast loads on the gpsimd (SWDGE) queue, xbar
# transposes on the sync HWDGE queue, x/w loads + output stores on the
# scalar HWDGE queue, so the streams never block each other. Every a
# strip and every aT chunk has a dedicated SBUF buffer (no pool
# recycling), so the whole DMA pipeline is dependency-free and streams
# at HBM rate.
#
# PE does nothing but real matmuls (~70us of back-to-back bf16 MMs), so
# the HAM clock-gate stays open naturally; a short warm-up burst (some
# tied to the first a-strip loads to spread it through the DMA window)
# covers the cold start.

import numpy as np

B, N, F, D = 8, 2048, 512, 512
P = 128
NT = N // P        # 16 row-tiles of n (and of m, since a is square)
FT = F // P        # 4 tiles of f
NCHUNK = 512       # n is processed in chunks of 512 columns
NJ = N // NCHUNK   # 4
NSUB = NCHUNK // P # 4

_CACHE = {}


def _build_nc():
    from contextlib import ExitStack

    from concourse import bacc, mybir, tile
    from concourse.masks import make_identity

    f32 = mybir.dt.float32
    bf16 = mybir.dt.bfloat16
    AF = mybir.ActivationFunctionType

    nc = bacc.Bacc(None)
    a_d = nc.dram_tensor("a", [N, N], f32, kind="ExternalInput")
    x_d = nc.dram_tensor("x", [N, F], f32, kind="ExternalInput")
    w_d = nc.dram_tensor("kernel", [F, D], f32, kind="ExternalInput")
    o_d = nc.dram_tensor("out", [N, D], f32, kind="ExternalOutput")

    with tile.TileContext(nc) as tc, ExitStack() as ctx:
        const = ctx.enter_context(tc.tile_pool(name="const", bufs=1))
        xp = ctx.enter_context(tc.tile_pool(name="xp", bufs=1))
        wp = ctx.enter_context(tc.tile_pool(name="wp", bufs=1))
        abp = ctx.enter_context(tc.tile_pool(name="abp", bufs=NT))
        atp = ctx.enter_context(tc.tile_pool(name="atp", bufs=NJ))
        ttp = ctx.enter_context(tc.tile_pool(name="ttp", bufs=2))
        outp = ctx.enter_context(tc.tile_pool(name="outp", bufs=3))
        scr = ctx.enter_context(tc.tile_pool(name="scr", bufs=2))
        ps_mm = ctx.enter_context(tc.tile_pool(name="ps_mm", bufs=4, space="PSUM"))
        ps_o = ctx.enter_context(tc.tile_pool(name="ps_o", bufs=2, space="PSUM"))
        ps_w = ctx.enter_context(tc.tile_pool(name="ps_w", bufs=2, space="PSUM"))

        ident = const.tile([P, P], bf16)
        make_identity(nc, ident[:])

        def warm(n, rhs=None):
            # bf16 identity matmuls: register as HAM activity, output unused.
            for _ in range(n):
                r = ident[:] if rhs is None else rhs
                nfree = r.shape[-1]
                pw = ps_w.tile([P, NCHUNK], f32, tag="psw", name="pw")
                nc.tensor.matmul(
                    pw[:, :nfree], lhsT=ident[:], rhs=r, start=True, stop=True
                )

        warm(12)

        # a: 16 strip cast-loads (fp32 HBM -> bf16 SBUF) on the gpsimd
        # (SWDGE) queue -- the only casting DMAs; each is 128 contiguous
        # 8KB-per-partition reads, so the SWDGE descriptor ring stays small.
        # All strips have dedicated buffers, so the loads stream
        # back-to-back at HBM rate with no recycle dependencies.
        ab = []
        for j in range(NT):
            t = abp.tile([P, N], bf16, tag="ab", name=f"ab{j}")
            nc.gpsimd.dma_start(t[:], a_d[j * P : (j + 1) * P, :])
            ab.append(t)
            if j < 4:
                # spread HAM warm-up through the load window: these warm
                # matmuls consume the just-loaded strip, so they execute
                # as the loads land instead of all at t=0.
                warm(6, rhs=t[:, 0:NCHUNK])

        # x, w: plain fp32 loads on the scalar HWDGE queue (no SWDGE ring
        # pressure), cast to bf16 on the idle DVE.
        x_sb = xp.tile([P, NT, F], bf16)
        for c in range(FT):
            xl = scr.tile([P, NT, P], f32, tag="load_scr", name=f"xl{c}")
            nc.scalar.dma_start(
                xl[:], x_d[:, c * P : (c + 1) * P].rearrange("(o p) f -> p o f", p=P)
            )
            nc.vector.tensor_copy(x_sb[:, :, c * P : (c + 1) * P], xl[:])

        w_sb = wp.tile([P, FT, D], bf16)
        wl = scr.tile([P, FT, D], f32, tag="wl")
        nc.scalar.dma_start(wl[:], w_d[:].rearrange("(o p) d -> p o d", p=P))
        nc.vector.tensor_copy(w_sb[:], wl[:])

        # aT chunks via DMA xbar transpose: strip j fills columns
        # [128*(j%4), 128*(j%4)+128) of chunk j//4 with
        # at[p, mi, jj] = strip[jj, 128*mi + p] = aT[128*mi+p, 128*j+jj]
        at = []
        for nj in range(NJ):
            t = atp.tile([P, NT, NCHUNK], bf16, tag="at", name=f"at{nj}")
            at.append(t)
        for j in range(NT):
            nc.sync.dma_start(
                at[j // 4][:, :, (j % 4) * P : (j % 4 + 1) * P],
                ab[j][:],
                transpose=True,
            )

        # mask accumulators; the per-row-tile |x| reductions ride along
        # inside chunk 0's mm1 phase.
        sumabs = const.tile([P, NT], f32)
        mask_sb = const.tile([P, NT], f32)

        cb = 0

        def copyback(dst, src, eng=None):
            nonlocal cb
            if eng is None:
                eng = "v" if cb % 2 == 0 else "s"
                cb += 1
            if eng == "v":
                nc.vector.tensor_copy(dst, src)
            else:
                nc.scalar.copy(dst, src)

        tts = []
        for nj in range(NJ):
            tt_sb = ttp.tile([P, FT, NCHUNK], bf16, tag="tt", name=f"tt{nj}")
            tts.append(tt_sb)
            for fi in range(FT):
                pt = ps_mm.tile([P, NCHUNK], f32, tag="psm", name=f"pt_{nj}_{fi}")
                for mi in range(NT):
                    nc.tensor.matmul(
                        pt[:],
                        lhsT=x_sb[:, mi, fi * P : (fi + 1) * P],
                        rhs=at[nj][:, mi, :],
                        start=(mi == 0),
                        stop=(mi == NT - 1),
                    )
                if nj == 0:
                    for ni in range(fi * 4, fi * 4 + 4):
                        abs_scr = scr.tile([P, F], f32, tag="abs_scr")
                        nc.scalar.activation(
                            abs_scr[:],
                            x_sb[:, ni, :],
                            AF.Abs,
                            accum_out=sumabs[:, ni : ni + 1],
                        )
                copyback(tt_sb[:, fi], pt[:])
            if nj == 0:
                nc.vector.tensor_scalar(
                    mask_sb[:], sumabs[:], 0.0, None, mybir.AluOpType.is_gt
                )

            # out rows for this chunk: accumulate over the 4 f-tiles, then
            # fused relu+mask on ACT, then store on the scalar HWDGE queue.
            for ns in range(NSUB):
                po = ps_o.tile([P, D], f32, tag="pso", name=f"po_{nj}_{ns}")
                for fi in range(FT):
                    nc.tensor.matmul(
                        po[:],
                        lhsT=tt_sb[:, fi, ns * P : (ns + 1) * P],
                        rhs=w_sb[:, fi],
                        start=(fi == 0),
                        stop=(fi == FT - 1),
                    )
                ni = nj * NSUB + ns
                ob = outp.tile([P, D], f32, tag="ob")
                nc.scalar.activation(
                    ob[:], po[:], AF.Relu, scale=mask_sb[:, ni : ni + 1]
                )
                nc.scalar.dma_start(o_d[ni * P : (ni + 1) * P, :], ob[:])

    nc.compile()
    return nc


def get_nc():
    if "nc" not in _CACHE:
        _CACHE["nc"] = _build_nc()
    return _CACHE["nc"]


def kernel(**inputs) -> np.ndarray:
    from concourse.bass_utils import run_bass_kernel_spmd

    x = np.ascontiguousarray(np.asarray(inputs["x"], dtype=np.float32))
    a = np.ascontiguousarray(np.asarray(inputs["a"], dtype=np.float32))
    w = np.ascontiguousarray(np.asarray(inputs["kernel"], dtype=np.float32))
    assert x.shape == (B, N, F) and a.shape == (B, N, N) and w.shape == (F, D)

    nc = get_nc()
    in_maps = [{"a": a[b], "x": x[b], "kernel": w} for b in range(B)]
    res = run_bass_kernel_spmd(nc, in_maps, core_ids=list(range(B)))
    return np.stack([res.results[b]["out"] for b in range(B)], axis=0)


# revision 6
# speedup vs baseline: 1.2560x; 1.1481x over previous
# GCN layer kernel for Trainium2: out[b] = relu((a[b] @ x[b]) @ W) * mask[b]
#
# Sharding: data-parallel over the batch (graph) dim. B=8 graphs, 8 cores,
# one graph per core; W replicated. Inputs are the FULL tensors; shards are
# sliced host-side and the per-core outputs stacked back together.
#
# Per-core dataflow (a: [2048,2048], x: [2048,512], W: [512,512]):
#   - All matmul operands are bf16 (same PE rate as f32r; rel-err ~3e-3 vs
#     the 2e-2 gate). a is cast fp32->bf16 during the DMA load itself
#     (SWDGE cast-on-DMA), so no engine time is spent on the big cast.
#   - a must be contracted over its column index m, which requires aT with
#     m on the partition axis. Instead of 256 PE transposes (~44us of PE
#     time), aT is produced by the DMA xbar transpose (SBUF->SBUF, bf16):
#     one dma_start(transpose=True) per 512-row group of a, writing the
#     whole aT chunk [128, 64, 128] in one instruction with both sides
#     contiguous per partition:
#       at[p, 16k+mi, j] = a[512g + 128k + j, 128mi + p]
#     mm1's rhs for (g, mi) is the strided slice at[g][:, mi::16, :]
#     ([128, 4, 128] AP = 512 n-columns in order).
#   - t^T[f,n] = sum_m x[m,f] * aT[m,n]:  lhsT = x tile, rhs = aT slice.
#   - out[n,d] = sum_f t^T[f,n] * W[f,d]: lhsT = t^T tile, rhs = W, which
#     lands out in [n,d] layout; 4 row-tiles batch into one 1MB store.
#   - mask[n] = any(x[n,:] != 0) via sum(|x|) > 0 on ACT, applied fused
#     into the ReLU (relu(mask * t) == mask * relu(t)).
#
# DMA design: the tile scheduler serializes DMAs through a small pool of
# completion semaphores, and every link in that chain costs the transfer
# plus ~2us of completion-receipt latency. So the kernel uses FEW, BIG
# DMAs: 4 a-group cast-loads (4MB read each, gpsimd/SWDGE queue), 4 xbar
# transposes (2MB SBUF->SBUF each, sync queue), 4 x column loads + 1 w
# load + 4 output stores (scalar queue) = 17 DMAs total.
#
# PE does nothing but real matmuls (~70us of back-to-back bf16 MMs), so
# the HAM clock-gate stays open naturally; warm-up bursts (some tied to
# the first loads to spread them through the DMA window) cover the start.

import numpy as np

B, N, F, D = 8, 2048, 512, 512
P = 128
NT = N // P        # 16 row-tiles of n (and of m, since a is square)
FT = F // P        # 4 tiles of f
NCHUNK = 512       # n is processed in chunks of 512 columns
NJ = N // NCHUNK   # 4
NSUB = NCHUNK // P # 4

_CACHE = {}


def _build_nc():
    from contextlib import ExitStack

    from concourse import bacc, mybir, tile
    from concourse.masks import make_identity

    f32 = mybir.dt.float32
    bf16 = mybir.dt.bfloat16
    AF = mybir.ActivationFunctionType

    nc = bacc.Bacc(None)
    a_d = nc.dram_tensor("a", [N, N], f32, kind="ExternalInput")
    x_d = nc.dram_tensor("x", [N, F], f32, kind="ExternalInput")
    w_d = nc.dram_tensor("kernel", [F, D], f32, kind="ExternalInput")
    o_d = nc.dram_tensor("out", [N, D], f32, kind="ExternalOutput")

    with tile.TileContext(nc) as tc, ExitStack() as ctx:
        const = ctx.enter_context(tc.tile_pool(name="const", bufs=1))
        xp = ctx.enter_context(tc.tile_pool(name="xp", bufs=1))
        wp = ctx.enter_context(tc.tile_pool(name="wp", bufs=1))
        abp = ctx.enter_context(tc.tile_pool(name="abp", bufs=3))
        atp = ctx.enter_context(tc.tile_pool(name="atp", bufs=NJ))
        ttp = ctx.enter_context(tc.tile_pool(name="ttp", bufs=2))
        outp = ctx.enter_context(tc.tile_pool(name="outp", bufs=2))
        scr = ctx.enter_context(tc.tile_pool(name="scr", bufs=2))
        ps_mm = ctx.enter_context(tc.tile_pool(name="ps_mm", bufs=4, space="PSUM"))
        ps_o = ctx.enter_context(tc.tile_pool(name="ps_o", bufs=2, space="PSUM"))
        ps_w = ctx.enter_context(tc.tile_pool(name="ps_w", bufs=2, space="PSUM"))

        ident = const.tile([P, P], bf16)
        make_identity(nc, ident[:])

        def warm(n, rhs=None):
            # bf16 identity matmuls: register as HAM activity, output unused.
            for _ in range(n):
                r = ident[:] if rhs is None else rhs
                nfree = r.shape[-1]
                pw = ps_w.tile([P, NCHUNK], f32, tag="psw", name="pw")
                nc.tensor.matmul(
                    pw[:, :nfree], lhsT=ident[:], rhs=r, start=True, stop=True
                )

        warm(12)

        # a: 4 group cast-loads (512 rows each, fp32 HBM -> bf16 SBUF) on
        # the gpsimd (SWDGE) queue: ab[p, k, m] = a[512g + 128k + p, m]
        x_sb = xp.tile([P, NT, F], bf16)
        ab = []
        for g in range(NJ):
            t = abp.tile([P, NJ, N], bf16, tag="ab", name=f"ab{g}")
            nc.gpsimd.dma_start(
                t[:], a_d[g * NCHUNK : (g + 1) * NCHUNK, :].rearrange(
                    "(k p) m -> p k m", p=P
                ),
            )
            ab.append(t)
            if g == 0:
                warm(8, rhs=t[:, 0, 0:NCHUNK])
            # x, w: plain fp32 loads on the scalar HWDGE queue, cast to
            # bf16 on the idle DVE. Column chunk fi feeds mm1's fi-th pass.
            if g < FT:
                xl = scr.tile([P, NT, P], f32, tag="load_scr", name=f"xl{g}")
                nc.scalar.dma_start(
                    xl[:],
                    x_d[:, g * P : (g + 1) * P].rearrange("(o p) f -> p o f", p=P),
                )
                nc.vector.tensor_copy(x_sb[:, :, g * P : (g + 1) * P], xl[:])
                if g < 2:
                    warm(5, rhs=x_sb[:, 0, g * P : (g + 1) * P])

        w_sb = wp.tile([P, FT, D], bf16)
        wl = scr.tile([P, FT, D], f32, tag="wl")
        nc.scalar.dma_start(wl[:], w_d[:].rearrange("(o p) d -> p o d", p=P))
        nc.vector.tensor_copy(w_sb[:], wl[:])

        # aT chunks via DMA xbar transpose, one per 512-row group:
        # at[p, e, j] = ab_flat[j, 128e + p], i.e.
        # at[p, 16k+mi, j] = a[512g + 128k + j, 128mi + p] = aT[.,.]
        at = []
        for g in range(NJ):
            t = atp.tile([P, NJ * NT, P], bf16, tag="at", name=f"at{g}")
            nc.sync.dma_start(t[:], ab[g][:], transpose=True)
            at.append(t)

        # mask accumulators; the per-row-tile |x| reductions ride along
        # inside chunk 0's mm1 phase.
        sumabs = const.tile([P, NT], f32)
        mask_sb = const.tile([P, NT], f32)

        cb = 0

        def copyback(dst, src, eng=None):
            nonlocal cb
            if eng is None:
                eng = "v" if cb % 2 == 0 else "s"
                cb += 1
            if eng == "v":
                nc.vector.tensor_copy(dst, src)
            else:
                nc.scalar.copy(dst, src)

        for nj in range(NJ):
            tt_sb = ttp.tile([P, FT, NCHUNK], bf16, tag="tt", name=f"tt{nj}")
            for fi in range(FT):
                pt = ps_mm.tile([P, NCHUNK], f32, tag="psm", name=f"pt_{nj}_{fi}")
                for mi in range(NT):
                    nc.tensor.matmul(
                        pt[:],
                        lhsT=x_sb[:, mi, fi * P : (fi + 1) * P],
                        rhs=at[nj][:, mi : NJ * NT : NT, :],
                        start=(mi == 0),
                        stop=(mi == NT - 1),
                    )
                if nj == 0:
                    for ni in range(fi * 4, fi * 4 + 4):
                        abs_scr = scr.tile([P, F], f32, tag="abs_scr")
                        nc.scalar.activation(
                            abs_scr[:],
                            x_sb[:, ni, :],
                            AF.Abs,
                            accum_out=sumabs[:, ni : ni + 1],
                        )
                copyback(tt_sb[:, fi], pt[:])
            if nj == 0:
                nc.vector.tensor_scalar(
                    mask_sb[:], sumabs[:], 0.0, None, mybir.AluOpType.is_gt
                )

            # out rows for this chunk: accumulate over the 4 f-tiles, then
            # fused relu+mask on ACT; 4 row-tiles batch into one 1MB store
            # on the scalar HWDGE queue.
            ob = outp.tile([P, NSUB, D], f32, tag="ob", name=f"ob{nj}")
            for ns in range(NSUB):
                po = ps_o.tile([P, D], f32, tag="pso", name=f"po_{nj}_{ns}")
                for fi in range(FT):
                    nc.tensor.matmul(
                        po[:],
                        lhsT=tt_sb[:, fi, ns * P : (ns + 1) * P],
                        rhs=w_sb[:, fi],
                        start=(fi == 0),
                        stop=(fi == FT - 1),
                    )
                ni = nj * NSUB + ns
                nc.scalar.activation(
                    ob[:, ns], po[:], AF.Relu, scale=mask_sb[:, ni : ni + 1]
                )
            nc.scalar.dma_start(
                o_d[nj * NCHUNK : (nj + 1) * NCHUNK, :].rearrange(
                    "(k p) d -> p k d", p=P
                ),
                ob[:],
            )

    nc.compile()
    return nc


def get_nc():
    if "nc" not in _CACHE:
        _CACHE["nc"] = _build_nc()
    return _CACHE["nc"]


def kernel(**inputs) -> np.ndarray:
    from concourse.bass_utils import run_bass_kernel_spmd

    x = np.ascontiguousarray(np.asarray(inputs["x"], dtype=np.float32))
    a = np.ascontiguousarray(np.asarray(inputs["a"], dtype=np.float32))
    w = np.ascontiguousarray(np.asarray(inputs["kernel"], dtype=np.float32))
    assert x.shape == (B, N, F) and a.shape == (B, N, N) and w.shape == (F, D)

    nc = get_nc()
    in_maps = [{"a": a[b], "x": x[b], "kernel": w} for b in range(B)]
    res = run_bass_kernel_spmd(nc, in_maps, core_ids=list(range(B)))
    return np.stack([res.results[b]["out"] for b in range(B)], axis=0)


# revision 7
# speedup vs baseline: 1.3357x; 1.0635x over previous
# GCN layer kernel for Trainium2: out[b] = relu((a[b] @ x[b]) @ W) * mask[b]
#
# Sharding: data-parallel over the batch (graph) dim. B=8 graphs, 8 cores,
# one graph per core; W replicated. Inputs are the FULL tensors; shards are
# sliced host-side and the per-core outputs stacked back together.
#
# Per-core dataflow (a: [2048,2048], x: [2048,512], W: [512,512]):
#   - All matmul operands are bf16 (same PE rate as f32r; rel-err ~3e-3 vs
#     the 2e-2 gate). a and W are cast fp32->bf16 during the DMA load
#     itself (SWDGE cast-on-DMA), x is cast on the mostly-idle DVE.
#   - a must be contracted over its column index m, which requires aT with
#     m on the partition axis. Instead of 256 PE transposes (~44us of PE
#     time), aT is produced by the DMA xbar transpose (SBUF->SBUF, bf16):
#     one dma_start(transpose=True) per row-group of a, writing
#       at[p, (k,mi), j] = a[nbase + 128k + j, 128mi + p]
#     with both sides contiguous per partition. mm1's rhs for (g, mi) is
#     at[g][:, :, mi, :] ([128, k, 128] = the chunk's n-columns in order).
#   - t^T[f,n] = sum_m x[m,f] * aT[m,n]:  lhsT = x tile, rhs = aT slice.
#   - out[n,d] = sum_f t^T[f,n] * W[f,d]: lhsT = t^T tile, rhs = W, which
#     lands out in [n,d] layout; 4 row-tiles batch into one 1MB store.
#   - mask[n] = any(x[n,:] != 0) via sum(|x|) > 0 on ACT, applied fused
#     into the ReLU (relu(mask * t) == mask * relu(t)).
#
# DMA schedule (learned from traces):
#   - The tile scheduler chains DMAs through a small completion-semaphore
#     pool and each chain link costs ~2us of completion-receipt latency,
#     so use FEW, BIG DMAs (~20 total).
#   - SWDGE (gpsimd) emission is non-blocking and one queue's descriptors
#     are consumed strictly FIFO, so all a-group loads are queued up front
#     on gpsimd: a0 completes first and each group follows continuously at
#     HBM rate. The first group is split in half so chunk 0's compute can
#     start ~10us earlier; mm1 processes chunk 0 as two 256-wide halves.
#   - HWDGE DMA instructions block their issuing engine while the ring is
#     full, so the sync queue carries ONLY the 5 transposes; x loads ride
#     the scalar queue early (ACT is idle then); stores ride gpsimd where
#     emission just waits for the ReLU data (engine idle after the loads).
#
# PE does nothing but real matmuls (~70us of back-to-back bf16 MMs), so
# the HAM clock-gate stays open naturally; warm-up bursts (some tied to
# the first loads to spread them through the DMA window) cover the start.

import numpy as np

B, N, F, D = 8, 2048, 512, 512
P = 128
NT = N // P        # 16 row-tiles of n (and of m, since a is square)
FT = F // P        # 4 tiles of f
NCHUNK = 512       # n is processed in chunks of 512 columns
NJ = N // NCHUNK   # 4
NSUB = NCHUNK // P # 4

_CACHE = {}


def _build_nc():
    from contextlib import ExitStack

    from concourse import bacc, mybir, tile
    from concourse.masks import make_identity

    f32 = mybir.dt.float32
    bf16 = mybir.dt.bfloat16
    AF = mybir.ActivationFunctionType

    nc = bacc.Bacc(None)
    a_d = nc.dram_tensor("a", [N, N], f32, kind="ExternalInput")
    x_d = nc.dram_tensor("x", [N, F], f32, kind="ExternalInput")
    w_d = nc.dram_tensor("kernel", [F, D], f32, kind="ExternalInput")
    o_d = nc.dram_tensor("out", [N, D], f32, kind="ExternalOutput")

    with tile.TileContext(nc) as tc, ExitStack() as ctx:
        const = ctx.enter_context(tc.tile_pool(name="const", bufs=1))
        xp = ctx.enter_context(tc.tile_pool(name="xp", bufs=1))
        wp = ctx.enter_context(tc.tile_pool(name="wp", bufs=1))
        abh = ctx.enter_context(tc.tile_pool(name="abh", bufs=2))
        abp = ctx.enter_context(tc.tile_pool(name="abp", bufs=3))
        ath = ctx.enter_context(tc.tile_pool(name="ath", bufs=2))
        atp = ctx.enter_context(tc.tile_pool(name="atp", bufs=3))
        ttp = ctx.enter_context(tc.tile_pool(name="ttp", bufs=2))
        outp = ctx.enter_context(tc.tile_pool(name="outp", bufs=2))
        scr = ctx.enter_context(tc.tile_pool(name="scr", bufs=2))
        ps_mm = ctx.enter_context(tc.tile_pool(name="ps_mm", bufs=4, space="PSUM"))
        ps_o = ctx.enter_context(tc.tile_pool(name="ps_o", bufs=2, space="PSUM"))
        ps_w = ctx.enter_context(tc.tile_pool(name="ps_w", bufs=2, space="PSUM"))

        ident = const.tile([P, P], bf16)
        make_identity(nc, ident[:])

        def warm(n, rhs=None):
            # bf16 identity matmuls: register as HAM activity, output unused.
            for _ in range(n):
                r = ident[:] if rhs is None else rhs
                nfree = r.shape[-1]
                pw = ps_w.tile([P, NCHUNK], f32, tag="psw", name="pw")
                nc.tensor.matmul(
                    pw[:, :nfree], lhsT=ident[:], rhs=r, start=True, stop=True
                )

        warm(12)

        # a loads on gpsimd (SWDGE cast fp32->bf16), consumed strictly FIFO:
        # group 0 as two 2-strip halves, then three 4-strip groups.
        # ab[p, k, m] = a[nbase + 128k + p, m]
        def load_a(nbase_rows, nrows, tag, pool, name):
            t = pool.tile([P, nrows // P, N], bf16, tag=tag, name=name)
            nc.gpsimd.dma_start(
                t[:],
                a_d[nbase_rows : nbase_rows + nrows, :].rearrange(
                    "(k p) m -> p k m", p=P
                ),
            )
            return t

        ab0a = load_a(0, 2 * P, "abh", abh, "ab0a")
        warm(6, rhs=ab0a[:, 0, 0:NCHUNK])
        ab0b = load_a(2 * P, 2 * P, "abh", abh, "ab0b")

        # w: SWDGE cast-load right behind the first a half-loads; lands by
        # ~20us, well before the first mm2 needs it.
        w_sb = wp.tile([P, FT, D], bf16)
        nc.gpsimd.dma_start(w_sb[:], w_d[:].rearrange("(o p) d -> p o d", p=P))

        ab = [None] * NJ
        for g in range(1, NJ):
            ab[g] = load_a(g * NCHUNK, NCHUNK, "ab", abp, f"ab{g}")

        # x: 4 column-chunk fp32 loads on the scalar HWDGE queue (ACT is
        # idle early), cast to bf16 on DVE. Chunk fi feeds mm1's fi-th pass.
        x_sb = xp.tile([P, NT, F], bf16)
        for c in range(FT):
            xl = scr.tile([P, NT, P], f32, tag="load_scr", name=f"xl{c}")
            nc.scalar.dma_start(
                xl[:], x_d[:, c * P : (c + 1) * P].rearrange("(o p) f -> p o f", p=P)
            )
            nc.vector.tensor_copy(x_sb[:, :, c * P : (c + 1) * P], xl[:])
            if c < 2:
                warm(5, rhs=x_sb[:, 0, c * P : (c + 1) * P])

        # aT via DMA xbar transpose on the sync queue (its only traffic):
        # at[p, k, mi, j] = a[nbase + 128k + j, 128mi + p]
        def transp(src, ksz, pool, tag, name):
            t = pool.tile([P, ksz, NT, P], bf16, tag=tag, name=name)
            nc.sync.dma_start(t[:], src[:], transpose=True)
            return t

        at0a = transp(ab0a, 2, ath, "ath", "at0a")
        at0b = transp(ab0b, 2, ath, "ath", "at0b")
        at = [None] * NJ
        for g in range(1, NJ):
            at[g] = transp(ab[g], 4, atp, "at", f"at{g}")

        # mask accumulators; the per-row-tile |x| reductions ride along
        # inside chunk 0's mm1 phase.
        sumabs = const.tile([P, NT], f32)
        mask_sb = const.tile([P, NT], f32)

        cb = 0

        def copyback(dst, src, eng=None):
            nonlocal cb
            if eng is None:
                eng = "v" if cb % 2 == 0 else "s"
                cb += 1
            if eng == "v":
                nc.vector.tensor_copy(dst, src)
            else:
                nc.scalar.copy(dst, src)

        for nj in range(NJ):
            tt_sb = ttp.tile([P, FT, NCHUNK], bf16, tag="tt", name=f"tt{nj}")
            for fi in range(FT):
                pt = ps_mm.tile([P, NCHUNK], f32, tag="psm", name=f"pt_{nj}_{fi}")
                if nj == 0:
                    # chunk 0 in two 256-wide halves so compute starts as
                    # soon as the first half-group transpose lands.
                    for h, ath_t in ((0, at0a), (1, at0b)):
                        sl = pt[:, h * 256 : (h + 1) * 256]
                        for mi in range(NT):
                            nc.tensor.matmul(
                                sl,
                                lhsT=x_sb[:, mi, fi * P : (fi + 1) * P],
                                rhs=ath_t[:, :, mi, :],
                                start=(mi == 0),
                                stop=(mi == NT - 1),
                            )
                    for ni in range(fi * 4, fi * 4 + 4):
                        abs_scr = scr.tile([P, F], f32, tag="abs_scr")
                        nc.scalar.activation(
                            abs_scr[:],
                            x_sb[:, ni, :],
                            AF.Abs,
                            accum_out=sumabs[:, ni : ni + 1],
                        )
                else:
                    for mi in range(NT):
                        nc.tensor.matmul(
                            pt[:],
                            lhsT=x_sb[:, mi, fi * P : (fi + 1) * P],
                            rhs=at[nj][:, :, mi, :],
                            start=(mi == 0),
                            stop=(mi == NT - 1),
                        )
                copyback(tt_sb[:, fi], pt[:])
            if nj == 0:
                nc.vector.tensor_scalar(
                    mask_sb[:], sumabs[:], 0.0, None, mybir.AluOpType.is_gt
                )

            # out rows for this chunk: accumulate over the 4 f-tiles, then
            # fused relu+mask on ACT; 4 row-tiles batch into one 1MB store
            # emitted from the (by now idle) gpsimd SWDGE queue.
            ob = outp.tile([P, NSUB, D], f32, tag="ob", name=f"ob{nj}")
            for ns in range(NSUB):
                po = ps_o.tile([P, D], f32, tag="pso", name=f"po_{nj}_{ns}")
                for fi in range(FT):
                    nc.tensor.matmul(
                        po[:],
                        lhsT=tt_sb[:, fi, ns * P : (ns + 1) * P],
                        rhs=w_sb[:, fi],
                        start=(fi == 0),
                        stop=(fi == FT - 1),
                    )
                ni = nj * NSUB + ns
                nc.scalar.activation(
                    ob[:, ns], po[:], AF.Relu, scale=mask_sb[:, ni : ni + 1]
                )
            nc.gpsimd.dma_start(
                o_d[nj * NCHUNK : (nj + 1) * NCHUNK, :].rearrange(
                    "(k p) d -> p k d", p=P
                ),
                ob[:],
            )

    nc.compile()
    return nc


def get_nc():
    if "nc" not in _CACHE:
        _CACHE["nc"] = _build_nc()
    return _CACHE["nc"]


def kernel(**inputs) -> np.ndarray:
    from concourse.bass_utils import run_bass_kernel_spmd

    x = np.ascontiguousarray(np.asarray(inputs["x"], dtype=np.float32))
    a = np.ascontiguousarray(np.asarray(inputs["a"], dtype=np.float32))
    w = np.ascontiguousarray(np.asarray(inputs["kernel"], dtype=np.float32))
    assert x.shape == (B, N, F) and a.shape == (B, N, N) and w.shape == (F, D)

    nc = get_nc()
    in_maps = [{"a": a[b], "x": x[b], "kernel": w} for b in range(B)]
    res = run_bass_kernel_spmd(nc, in_maps, core_ids=list(range(B)))
    return np.stack([res.results[b]["out"] for b in range(B)], axis=0)
